# revision 5
# baseline (speedup 1.0000x reference)
"""MoE layer kernel for Trainium2 (8 NeuronCores, SPMD via bass/Tile).

Strategy:
  - Host: gate (global-avg-pool -> Linear -> softmax -> top-2). Only the
    top-2 experts per sample contribute to the output (exp_w is zero
    elsewhere), so we compute just those: 16 (sample, expert) pairs.
  - Device: core b processes sample b with its 2 selected experts.
    out = x + sum_e (s_e * W2_e)^T gelu(W1_e^T x + b1_e)
    where s_e = topk_w[b,e] * k[b] is folded into W2 on the host.
    The b2 contribution (sum_e s_e*b2_e, a per-channel constant) is added
    on the host afterwards.
  - Layout: contraction dims (C=512, Dh=1024) are striped over the 128
    SBUF partitions; HW=1024 is the matmul moving dim (2 x 512).
"""

import os
import numpy as np

P = 128
C = 512
DH = 1024
HW = 1024
CO = C // P     # 4 chunks of C on partitions
DO = DH // P    # 8 chunks of Dh on partitions
NF = 512        # matmul moving-dim tile
NH = HW // NF   # 2
E2 = 2          # experts per sample (top-k)
B = 8

# matmul input dtype: "float32" (4 cyc/row) or "float32r" (1 cyc/row at N>=256)
MM_DTYPE = os.environ.get("MOE_MM_DTYPE", "float32r")

_NC_CACHE = {}


def _build_nc(mm_dtype_name):
    import concourse.mybir as mybir
    import concourse.tile as tile
    from concourse import bacc

    fp32 = mybir.dt.float32
    mmdt = getattr(mybir.dt, mm_dtype_name)

    nc = bacc.Bacc("TRN2", target_bir_lowering=False, debug=False, num_devices=B)

    x_d = nc.dram_tensor("x", [C, HW], mmdt, kind="ExternalInput")
    w1_d = nc.dram_tensor("w1", [E2, C, DH], mmdt, kind="ExternalInput")
    b1_d = nc.dram_tensor("b1", [E2, DH], fp32, kind="ExternalInput")
    w2_d = nc.dram_tensor("w2", [E2, DH, C], mmdt, kind="ExternalInput")
    out_d = nc.dram_tensor("out", [C, HW], fp32, kind="ExternalOutput")

    with tile.TileContext(nc) as tc:
        with (
            tc.tile_pool(name="const", bufs=1) as cpool,
            tc.tile_pool(name="psh", bufs=4, space="PSUM") as ph_pool,
            tc.tile_pool(name="psy", bufs=4, space="PSUM") as py_pool,
            tc.tile_pool(name="outp", bufs=4) as opool,
        ):
            x_sb = cpool.tile([P, CO, HW], mmdt)
            w1_sb = cpool.tile([P, E2, CO, DH], mmdt)
            b1_sb = cpool.tile([P, E2, DO], fp32)
            w2_sb = cpool.tile([P, E2, DO, C], mmdt)
            h_sb = cpool.tile([P, E2, DO, HW], mmdt)

            nc.sync.dma_start(x_sb[:], x_d.ap().rearrange("(o p) f -> p o f", p=P))
            nc.sync.dma_start(
                w1_sb[:], w1_d.ap().rearrange("e (o p) d -> p e o d", p=P)
            )
            nc.sync.dma_start(b1_sb[:], b1_d.ap().rearrange("e (o p) -> p e o", p=P))
            nc.sync.dma_start(
                w2_sb[:], w2_d.ap().rearrange("e (o p) c -> p e o c", p=P)
            )

            # Stage A: h[e] = gelu(W1_e^T x + b1_e)   (partitions: Dh chunk)
            for half in range(NH):
                hw_sl = slice(half * NF, (half + 1) * NF)
                for e in range(E2):
                    for do in range(DO):
                        ps = ph_pool.tile([P, NF], fp32, tag="ps_h")
                        for co in range(CO):
                            nc.tensor.matmul(
                                ps[:],
                                w1_sb[:, e, co, do * P:(do + 1) * P],
                                x_sb[:, co, hw_sl],
                                start=(co == 0),
                                stop=(co == CO - 1),
                            )
                        nc.scalar.activation(
                            h_sb[:, e, do, hw_sl],
                            ps[:],
                            mybir.ActivationFunctionType.Gelu,
                            bias=b1_sb[:, e, do:do + 1],
                            scale=1.0,
                        )

            # Stage B: out = x + sum_e (s_e W2_e)^T h_e  (partitions: C chunk)
            out_r = out_d.ap().rearrange("(o p) f -> p o f", p=P)
            for half in range(NH):
                hw_sl = slice(half * NF, (half + 1) * NF)
                for co in range(CO):
                    ps = py_pool.tile([P, NF], fp32, tag="ps_y")
                    n_acc = E2 * DO
                    i = 0
                    for e in range(E2):
                        for do in range(DO):
                            nc.tensor.matmul(
                                ps[:],
                                w2_sb[:, e, do, co * P:(co + 1) * P],
                                h_sb[:, e, do, hw_sl],
                                start=(i == 0),
                                stop=(i == n_acc - 1),
                            )
                            i += 1
                    ot = opool.tile([P, NF], fp32, tag="out_t")
                    nc.vector.tensor_add(ot[:], ps[:], x_sb[:, co, hw_sl].bitcast(fp32))
                    nc.sync.dma_start(out_r[:, co, hw_sl], ot[:])

    nc.compile()
    return nc


def _get_nc():
    if MM_DTYPE not in _NC_CACHE:
        _NC_CACHE[MM_DTYPE] = _build_nc(MM_DTYPE)
    return _NC_CACHE[MM_DTYPE]


def _gate(inputs, k, Wg, bg):
    """Replicates the reference gate in fp32 numpy."""
    Bn = inputs.shape[0]
    pooled = inputs.mean(axis=(2, 3), dtype=np.float32)       # [B, C]
    logits = pooled.astype(np.float32) @ Wg.astype(np.float32) + bg  # [B, E]
    m = logits.max(axis=1, keepdims=True)
    ew = np.exp(logits - m)
    sm = ew / ew.sum(axis=1, keepdims=True)                   # [B, E] softmax
    idx = np.argsort(-sm, axis=1, kind="stable")[:, :E2]      # [B, 2]
    topw = np.take_along_axis(sm, idx, axis=1)                # [B, 2]
    s = (topw * k.reshape(Bn, 1)).astype(np.float32)          # [B, 2]
    return idx, s


def kernel(inputs, k, Wg, bg, W1, b1, W2, b2):
    from concourse.bass_utils import run_bass_kernel_spmd

    inputs = np.asarray(inputs)
    Bn, Cn, Hn, Wn = inputs.shape
    idx, s = _gate(inputs, k, np.asarray(Wg), np.asarray(bg))

    x = np.ascontiguousarray(inputs.reshape(Bn, Cn, Hn * Wn)).astype(np.float32)
    W1 = np.asarray(W1, dtype=np.float32)
    b1 = np.asarray(b1, dtype=np.float32)
    W2 = np.asarray(W2, dtype=np.float32)
    b2 = np.asarray(b2, dtype=np.float32)

    in_maps = []
    for b in range(Bn):
        sel = idx[b]
        w2s = W2[sel] * s[b, :, None, None]                   # [2, Dh, C]
        in_maps.append({
            "x": x[b],
            "w1": np.ascontiguousarray(W1[sel]),
            "b1": np.ascontiguousarray(b1[sel]),
            "w2": np.ascontiguousarray(w2s.astype(np.float32)),
        })

    nc = _get_nc()
    res = run_bass_kernel_spmd(nc, in_maps, core_ids=list(range(Bn)))
    out = np.stack([res.results[b]["out"] for b in range(Bn)], axis=0)  # [B,C,HW]

    # b2 contribution: per-sample per-channel constant (zero in practice)
    bias_comb = np.einsum("bk,bkc->bc", s, b2[idx])           # [B, C]
    out = out + bias_comb[:, :, None]
    return out.reshape(Bn, Cn, Hn, Wn).astype(np.float32)
